# revision 2
# baseline (speedup 1.0000x reference)
"""MinLSTM Trainium2 kernel v3 (8-core data-parallel over batch).

Math (per batch):
  preacts: F = x@Wf.T+bf, I = x@Wi.T+bi, Hp = x@Wh.T+bh      [T, H]
  sf=sigmoid(F), si=sigmoid(I)
  f_gate = sf/(sf+si)  (normalized gates; f+i=1)
  g(z) = max(sigmoid(z), z+0.5)
  h[0] = g(h_0);  h[t] = f_gate[t]*h[t-1] + (1-f_gate[t])*g(Hp[t])

HW mapping per core (1 batch), from measured engine rates:
  - PE (bottleneck ~114us busy): F/I preacts fp8e4m3 DoubleRow (4 insts
    each per [128,512] tile, W host-prescaled x256), Hp fp16 (8 insts).
    Every matmul streams 518 cycles; LDWEIGHTS ride the reorder window.
  - ACT: 4 ops per tile reading PSUM with fused per-partition bias:
    sf/si (fp32, feed the fp32-only GATE_DIV bit-trick), sh=sigm(Hp+bh)
    and u=Hp+bh+0.5 (both bf16, so gg can be a 2x-mode bf16 TT).
  - DVE ops sized [128,512] so the per-time-chunk DVE load (~3.2us/tile)
    tracks the PE (~3.46us/tile) window-by-window:
      fg   = GATE_DIV_ML(sf,si): custom 7-stage op = sf/(sf+si) via
             BITWISE_NOT reciprocal seed + 1 Newton step (err ~2e-3)
      gg   = max(u, sh)  bf16 tensor_tensor (2x mode)
      nv   = stt(fg,-1,*gg)
      scan = tensor_tensor_scan(fg, nv)  bf16
    nv/scan are emitted 1/2 tiles behind fg/gg so each op's inputs were
    written >=2 ops earlier (no in-order write-ack stalls).
  - Startup-critical DMAs (w8[0] per-gate halves, x8 kd-halves) spread
    across scalar+sync+vector hwdge queues; consts on gpsimd SWDGE.
  - Output y is [H, T+1] bf16 h-major; host transposes during gather.
"""
import sys

sys.path.insert(0, "/opt/trn_rl_repo")
import numpy as np

B, T, D, H = 8, 2048, 1024, 1024
N_CORES = 8
P = 128
TCH = 512
N_TC = T // TCH        # 4 time chunks
HB = H // P            # 8 h blocks
KD = D // P            # 8 contraction blocks
KH = KD // 2           # k blocks per x half tile

GATE_DIV_C0 = -0.23549792
GATE_DIV_C1 = 2.0017324

_cache = {}


def _register_gate_div():
    from concourse import dve_ops
    from concourse.dve_spec import Spec, Src0, Src1, C0, C1, AluOp, Bin, lower
    from concourse.dve_uop import DveOpSpec

    name = "GATE_DIV_ML"
    for op in dve_ops.OPS:
        if op.name == name:
            return op

    d = Src0 + Src1
    nd = Bin(AluOp.BITWISE_NOT, d, d)
    y0 = nd * C0
    y1 = y0 * (C1 - d * y0)
    body = y1 * Src0

    def ref(in0, in1, c0, c1, c2):
        dd = (in0 + in1).astype(np.float32)
        nx = (~dd.view(np.int32)).view(np.float32)
        yy0 = nx * np.float32(c0)
        yy1 = yy0 * (np.float32(c1) - dd * yy0)
        return yy1 * in0

    spec = Spec(body=body, reference=ref)
    row = dve_ops._CUSTOM_DVE_ROW_BASE + len(dve_ops.OPS)
    dve_ops._SUB_OPCODE_FOR_NAME[name] = row
    sha = DveOpSpec(name=name, opcode=row, uops=lower(spec, ver="v3"),
                    rd1_en=True).sha("v3")
    op = dve_ops.DveOp(name, spec, subdim=False, uops_sha={"v3": sha})
    dve_ops.OPS.append(op)
    dve_ops.CUSTOM_DVE_SPECS[name] = spec
    return op


def _build_nc():
    import concourse.bacc as bacc
    import concourse.tile as tile
    from concourse import mybir
    from contextlib import ExitStack

    gate_div = _register_gate_div()

    fp32 = mybir.dt.float32
    fp16 = mybir.dt.float16
    fp8 = mybir.dt.float8e4
    bf16 = mybir.dt.bfloat16
    DR = mybir.MatmulPerfMode.DoubleRow
    ACT = mybir.ActivationFunctionType
    ALU = mybir.AluOpType

    nc = bacc.Bacc("TRN2", target_bir_lowering=False, debug=False,
                   num_devices=N_CORES)

    x8a = nc.dram_tensor("x8a", [N_TC, P, KH, TCH], fp8,
                         kind="ExternalInput")
    x8b = nc.dram_tensor("x8b", [N_TC, P, KH, TCH], fp8,
                         kind="ExternalInput")
    # fp16 x only for kd 2..7 (Hp's kd 0..1 ride the fp8 x via DoubleRow)
    xFa = nc.dram_tensor("xFa", [N_TC, P, 3, TCH], fp16,
                         kind="ExternalInput")
    xFb = nc.dram_tensor("xFb", [N_TC, P, 3, TCH], fp16,
                         kind="ExternalInput")
    h0 = nc.dram_tensor("h0", [1, H], fp32, kind="ExternalInput")
    wF = nc.dram_tensor("wF", [HB, P, 6, P], fp16, kind="ExternalInput")
    w8 = nc.dram_tensor("w8", [HB, P, KD, 2 * P], fp8,
                        kind="ExternalInput")
    wh8 = nc.dram_tensor("wh8", [P, HB, 2, P], fp8, kind="ExternalInput")
    bft = nc.dram_tensor("bf", [H], fp32, kind="ExternalInput")
    bit = nc.dram_tensor("bi", [H], fp32, kind="ExternalInput")
    bht = nc.dram_tensor("bh", [H], fp32, kind="ExternalInput")
    y = nc.dram_tensor("y", [H, T + 1], bf16, kind="ExternalOutput")

    with tile.TileContext(nc) as tc:
        with ExitStack() as ctx:
            consts = ctx.enter_context(tc.tile_pool(name="consts", bufs=1))
            wt_pool = ctx.enter_context(tc.tile_pool(name="wt", bufs=1))
            xt_pool = ctx.enter_context(tc.tile_pool(name="xt", bufs=3))
            sg_pool = ctx.enter_context(tc.tile_pool(name="sg", bufs=3))
            fg_pool = ctx.enter_context(tc.tile_pool(name="fg", bufs=4))
            hs_pool = ctx.enter_context(tc.tile_pool(name="hs", bufs=2))
            mm_ps = ctx.enter_context(
                tc.tile_pool(name="mmps", bufs=8, space="PSUM"))

            # ---- constants: biases, h0 (gpsimd SWDGE keeps them off the
            # hwdge queues that feed the startup x/W streams) ----
            def load_col(name, src_ap):
                t = consts.tile([P, HB], fp32, name=name)
                nc.gpsimd.dma_start(
                    out=t, in_=src_ap.rearrange("(hb p) -> p hb", p=P))
                return t

            bf_t = load_col("bf_t", bft[:])
            bi_t = load_col("bi_t", bit[:])
            bh_t = load_col("bh_t", bht[:])
            h0_t = load_col("h0_t", h0[0, :])

            bhp5 = consts.tile([P, HB], fp32, name="bhp5")
            nc.vector.tensor_scalar_add(bhp5, bh_t, 0.5)
            sh0 = consts.tile([P, HB], fp32, name="sh0")
            nc.scalar.activation(sh0, h0_t, ACT.Sigmoid)
            g0 = consts.tile([P, HB], fp32, name="g0")
            # g0 = max(h0 + 0.5, sigmoid(h0))
            nc.vector.scalar_tensor_tensor(g0, h0_t, 0.5, sh0,
                                           op0=ALU.add, op1=ALU.max)
            g0b = consts.tile([P, HB], bf16, name="g0b")
            nc.vector.tensor_copy(g0b, g0)
            nc.gpsimd.dma_start(
                out=y[:, 0:1].rearrange("(hb p) one -> p (hb one)", p=P),
                in_=g0b)

            # steady-state input DMAs alternate scalar/sync hwdge queues
            qctr = [0]

            def in_dma(**kw):
                (nc.scalar if qctr[0] % 2 == 0 else nc.sync).dma_start(**kw)
                qctr[0] += 1

            wft = [None] * HB
            w8t = [None] * HB

            def emit_w_dma(hb):
                t8 = wt_pool.tile([P, KD, 2 * P], fp8, name=f"w8t{hb}")
                in_dma(out=t8, in_=w8[hb])
                w8t[hb] = t8
                t = wt_pool.tile([P, 6, P], fp16, name=f"wft{hb}")
                in_dma(out=t, in_=wF[hb])
                wft[hb] = t

            def emit_x_dma(tci):
                # steady-state x chunks ride the (otherwise idle) SWDGE
                # queue: their triggers never queue behind ACT work, and
                # the hwdge queues keep their bandwidth for weights + y
                t8a = xt_pool.tile([P, KH, TCH], fp8,
                                   name=f"x8a_{tci}", tag="x8a")
                nc.gpsimd.dma_start(out=t8a, in_=x8a[tci])
                t8b = xt_pool.tile([P, KH, TCH], fp8,
                                   name=f"x8b_{tci}", tag="x8b")
                nc.gpsimd.dma_start(out=t8b, in_=x8b[tci])
                tfa = xt_pool.tile([P, 3, TCH], fp16,
                                   name=f"xfa_{tci}", tag="xfa")
                nc.gpsimd.dma_start(out=tfa, in_=xFa[tci])
                tfb = xt_pool.tile([P, 3, TCH], fp16,
                                   name=f"xfb_{tci}", tag="xfb")
                nc.gpsimd.dma_start(out=tfb, in_=xFb[tci])
                return (t8a, t8b), (tfa, tfb)

            wh8_t = consts.tile([P, HB, 2, P], fp8, name="wh8_t")

            def emit_startup_dmas():
                # tile (hb=0, tci=0) needs, in order: w8[0] gate-0 half +
                # x8a[0] (first 2 F matmuls), x8b[0], w8[0] gate-1 half,
                # wh8, then wF[0] + xF[0] pieces. Spread across the two
                # hwdge queues + the gpsimd SWDGE queue.
                t8 = wt_pool.tile([P, KD, 2 * P], fp8, name="w8t0")
                w8t[0] = t8
                nc.scalar.dma_start(out=t8[:, :, 0:P], in_=w8[0, :, :, 0:P])
                t8a = xt_pool.tile([P, KH, TCH], fp8, name="x8a_0",
                                   tag="x8a")
                nc.sync.dma_start(out=t8a, in_=x8a[0])
                t8b = xt_pool.tile([P, KH, TCH], fp8, name="x8b_0",
                                   tag="x8b")
                nc.gpsimd.dma_start(out=t8b, in_=x8b[0])
                nc.scalar.dma_start(out=t8[:, :, P:2 * P],
                                    in_=w8[0, :, :, P:2 * P])
                nc.scalar.dma_start(out=wh8_t, in_=wh8[:])
                tf = wt_pool.tile([P, 6, P], fp16, name="wft0")
                wft[0] = tf
                nc.sync.dma_start(out=tf, in_=wF[0])
                tfa = xt_pool.tile([P, 3, TCH], fp16, name="xfa_0",
                                   tag="xfa")
                nc.sync.dma_start(out=tfa, in_=xFa[0])
                tfb = xt_pool.tile([P, 3, TCH], fp16, name="xfb_0",
                                   tag="xfb")
                nc.scalar.dma_start(out=tfb, in_=xFb[0])
                return (t8a, t8b), (tfa, tfb)

            prev_hs = {}
            nv_q = []      # (hb, tci, fg, gg)
            scan_q = []    # (hb, tci, fg, nv)

            def emit_nv():
                hb, tci, fg, gg = nv_q.pop(0)
                nvt = fg_pool.tile([P, TCH], bf16,
                                   name=f"nv{hb}_{tci}", tag="nv")
                # nv = (f-1)*g  (scan's op1=subtract adds (1-f)*g)
                nc.vector.scalar_tensor_tensor(
                    nvt, fg, 1.0, gg, op0=ALU.subtract, op1=ALU.mult)
                scan_q.append((hb, tci, fg, nvt))

            def emit_scan():
                hb, tci, fg, nvt = scan_q.pop(0)
                hs = hs_pool.tile([P, TCH], bf16,
                                  name=f"hs{hb}_{tci}", tag=f"hs{hb}")
                init = (g0[:, hb:hb + 1] if tci == 0
                        else prev_hs[hb][:, TCH - 1:TCH])
                nc.vector.tensor_tensor_scan(hs, fg, nvt, init,
                                             op0=ALU.mult,
                                             op1=ALU.subtract)
                prev_hs[hb] = hs
                t0 = tci * TCH
                # split y-out bandwidth across both hwdge queues
                eng = nc.scalar if hb % 2 == 0 else nc.sync
                eng.dma_start(
                    out=y[hb * P:(hb + 1) * P, 1 + t0:1 + t0 + TCH],
                    in_=hs)

            def emit_tile(hb, tci, x8t, xft):
                pf = mm_ps.tile([P, TCH], fp32, name=f"pf{hb}_{tci}",
                                tag="mm")
                pi = mm_ps.tile([P, TCH], fp32, name=f"pi{hb}_{tci}",
                                tag="mm")
                ph = mm_ps.tile([P, TCH], fp32, name=f"ph{hb}_{tci}",
                                tag="mm")
                for g, psg in ((0, pf), (1, pi)):
                    for half in range(2):
                        for k2 in range(KH // 2):
                            kd = half * KH + 2 * k2
                            nc.tensor.matmul(
                                psg,
                                w8t[hb][:, kd:kd + 2, g * P:(g + 1) * P],
                                x8t[half][:, 2 * k2:2 * k2 + 2, :],
                                start=(half == 0 and k2 == 0),
                                stop=(half == 1 and k2 == KH // 2 - 1),
                                perf_mode=DR)
                # Hp: kd 0..1 in one fp8 DoubleRow inst, kd 2..7 fp16
                nc.tensor.matmul(ph, wh8_t[:, hb], x8t[0][:, 0:2, :],
                                 start=True, stop=False, perf_mode=DR)
                for xi in range(2):
                    for k in range(3):
                        nc.tensor.matmul(
                            ph, wft[hb][:, xi * 3 + k, :],
                            xft[xi][:, k, :],
                            start=False, stop=(xi == 1 and k == 2))

                sf = sg_pool.tile([P, TCH], fp32, name=f"sf{hb}_{tci}",
                                  tag="sf")
                si = sg_pool.tile([P, TCH], fp32, name=f"si{hb}_{tci}",
                                  tag="si")
                sh = sg_pool.tile([P, TCH], bf16, name=f"sh{hb}_{tci}",
                                  tag="sh")
                ut = sg_pool.tile([P, TCH], bf16, name=f"ut{hb}_{tci}",
                                  tag="ut")
                nc.scalar.activation(sf, pf, ACT.Sigmoid,
                                     bias=bf_t[:, hb:hb + 1],
                                     scale=1.0 / 256.0)
                nc.scalar.activation(si, pi, ACT.Sigmoid,
                                     bias=bi_t[:, hb:hb + 1],
                                     scale=1.0 / 256.0)
                # Hp weights are host-prescaled x256 (fp8 part needs it);
                # the ACT scale undoes it for both consumers
                nc.scalar.activation(sh, ph, ACT.Sigmoid,
                                     bias=bh_t[:, hb:hb + 1],
                                     scale=1.0 / 256.0)
                # u = Hp + bh + 0.5 (so gg below is an all-bf16 2x-mode TT)
                nc.scalar.activation(ut, ph, ACT.Identity,
                                     bias=bhp5[:, hb:hb + 1],
                                     scale=1.0 / 256.0)

                fg = fg_pool.tile([P, TCH], bf16, name=f"fg{hb}_{tci}",
                                  tag="fg")
                gg = fg_pool.tile([P, TCH], bf16, name=f"gg{hb}_{tci}",
                                  tag="gg")
                nc.vector._custom_dve(gate_div, out=fg, in0=sf, in1=si,
                                      s0=GATE_DIV_C0, s1=GATE_DIV_C1,
                                      imm2=0.0)
                # gg = max(Hp + bh + 0.5, sigmoid(Hp + bh))
                nc.vector.tensor_max(gg, ut, sh)
                nv_q.append((hb, tci, fg, gg))
                # one-behind pipeline: nv(k-1), scan(k-2) at tile k
                if len(nv_q) > 1:
                    emit_nv()
                if len(scan_q) > 1:
                    emit_scan()

            x_tiles = [None] * N_TC
            x_tiles[0] = emit_startup_dmas()
            for hb in range(1, HB):
                emit_w_dma(hb)

            for tci in range(N_TC):
                # prefetch next chunk a full time-chunk ahead (SWDGE)
                if tci + 1 < N_TC:
                    x_tiles[tci + 1] = emit_x_dma(tci + 1)
                x8t, xft = x_tiles[tci]
                for hb in range(HB):
                    emit_tile(hb, tci, x8t, xft)
            while nv_q:
                emit_nv()
            while scan_q:
                emit_scan()

    nc.compile()
    return nc


def _get_nc():
    if "nc" not in _cache:
        _cache["nc"] = _build_nc()
    return _cache["nc"]


def _run(inputs, trace=False, **kw):
    import ml_dtypes
    from concourse.bass_utils import run_bass_kernel_spmd

    nc = _get_nc()
    f8 = ml_dtypes.float8_e4m3
    # [b, tc, p, kd, t] = x[b, tc*TCH+t, kd*P+p]
    xTf = np.asarray(inputs["x"], dtype=np.float32).transpose(0, 2, 1)
    xR = xTf.reshape(B, KD, P, N_TC, TCH).transpose(0, 3, 2, 1, 4)
    xF = xR.astype(np.float16)
    x8 = xR.astype(f8)
    x8ah = np.ascontiguousarray(x8[:, :, :, :KH])
    x8bh = np.ascontiguousarray(x8[:, :, :, KH:])
    xFah = np.ascontiguousarray(xF[:, :, :, 2:5])
    xFbh = np.ascontiguousarray(xF[:, :, :, 5:8])
    h_0 = np.ascontiguousarray(inputs["h_0"], dtype=np.float32)
    ws = [np.asarray(inputs[k], dtype=np.float32) for k in
          ("Wf", "Wi", "Wh")]
    # whole Hp weight path prescaled x256 (fp8 kd 0..1 + fp16 kd 2..7);
    # the ACT sigmoid/identity descale by 1/256
    whT = (ws[2].T * 256.0).reshape(KD, P, HB, P)
    # wF[hb, p, k, m] = 256*Wh[hb*P+m, (k+2)*P+p]  (fp16)
    wFh = np.ascontiguousarray(
        whT[2:].transpose(2, 1, 0, 3).astype(np.float16))
    # wh8[p, hb, j, m] = 256*Wh[hb*P+m, j*P+p]  (fp8, kd 0..1)
    wh8h = np.ascontiguousarray(whT[:2].transpose(1, 2, 0, 3).astype(f8))
    # w8[hb, p, kd, g*P+m] = 256 * Wg[hb*P+m, kd*P+p]  (fp8, F/I gates)
    w8h = np.empty((HB, P, KD, 2 * P), dtype=f8)
    for g in range(2):
        t = (ws[g].T * 256.0).reshape(KD, P, HB, P).transpose(2, 1, 0, 3)
        w8h[:, :, :, g * P:(g + 1) * P] = t.astype(f8)
    w8h = np.ascontiguousarray(w8h)
    shared = {
        "wF": wFh,
        "w8": w8h,
        "wh8": wh8h,
        "bf": np.ascontiguousarray(inputs["bf"], dtype=np.float32),
        "bi": np.ascontiguousarray(inputs["bi"], dtype=np.float32),
        "bh": np.ascontiguousarray(inputs["bh"], dtype=np.float32),
    }
    in_maps = []
    for b in range(B):
        m = {"x8a": x8ah[b], "x8b": x8bh[b], "xFa": xFah[b],
             "xFb": xFbh[b], "h0": h_0[b], **shared}
        in_maps.append(m)
    res = run_bass_kernel_spmd(nc, in_maps, list(range(N_CORES)),
                               trace=trace, **kw)
    out = np.stack(
        [np.ascontiguousarray(
            np.asarray(res.results[b]["y"]).astype(np.float32).T)
         for b in range(B)], axis=0)
    return out, res


def kernel(**inputs) -> np.ndarray:
    out, _ = _run(inputs, trace=False)
    return out
